# revision 23
# baseline (speedup 1.0000x reference)
"""Trainium2 Bass kernel for a 2-layer GRU (H=10) + linear head.

Strategy (pure data parallel, 8 cores):
  - Shard batch B=1024 -> 128 per core; replicate the tiny weights.
  - Per core, the T=2048 recurrence runs as one fused chain covering BOTH
    GRU layers (layer 1 lags layer 0 by one step, software-pipelined).
  - fp16 state/weights: PE runs at 1 cycle/row (fp32 is 4).
  - PSUM-accumulated recurrence: h' = vv + uu  (vv = n*(1-z), uu = h*z,
    both on Pool), and W @ h' = W@vv + W@uu, so the step-(k+1)
    preactivations are built by THREE accumulating matmuls into one PSUM
    bank: mm_x (rank-1 x side, start=True), mm_uu, mm_vv.  The h'
    materialization (Pool add into the state chunk tile) is OFF the
    critical path - the next matmul fires as soon as vv lands.
  - TWO staggered streams per core (batch cols 0:64 / 64:128).  Each
    stream has its own PSUM banks + semaphores and runs the chain
    independently; engines interleave A/B instructions so the ~1.4us
    per-stream latency chain overlaps.  Throughput is ACT-bound
    (4 activation instrs/step).
  - PSUM layout per stream ([116, 64] fp32): z 0:20 | r 32:52 |
    hn 64:84 | xn 96:116 (20-row blocks at 32-row quadrant bases).
  - h' states accumulate in [33, CHUNK*128] fp16 chunk tiles (rows 0:20
    h0|h1, row 32 = x_t DMA'd in); layer-1 rows DMA out per chunk; the
    final linear head (10 -> 1) runs on host.
"""

import numpy as np
F16 = np.float16

H = 10
B = 1024
T = 2048
NCORES = 8
BL = B // NCORES  # 128 batch rows per core
NS = 3            # staggered streams per core
_W = [BL // NS + (1 if i < BL % NS else 0) for i in range(NS)]
_C0 = [sum(_W[:i]) for i in range(NS)]    # col offset of stream i
CHUNK = 64        # time slots per SBUF state chunk
SR = 33           # state rows: h0(10) | h1(10) | pad | x @ 32
XROW = 32         # x row partition (matmul base partition must be 0/32/64)

_PROGRAM_CACHE = {}


def _dims(t_steps):
    nstep = t_steps + 1          # macro-steps 0..t (layer1 lags by one)
    nslot = nstep + 1            # state slots
    nch = (nslot + CHUNK - 1) // CHUNK
    return nstep, nslot, nch


def _build_program(t_steps):
    from contextlib import ExitStack

    import concourse.bass as bass
    import concourse.mybir as mybir

    fp32 = mybir.dt.float32
    f16 = mybir.dt.float16
    Alu = mybir.AluOpType
    Act = mybir.ActivationFunctionType

    nstep, nslot, nch = _dims(t_steps)

    nc = bass.Bass()

    x_d = nc.declare_dram_parameter("xt", [nch, CHUNK * BL], f16,
                                    isOutput=False)
    wrec_d = nc.declare_dram_parameter("w_rec", [SR, 116], f16,
                                       isOutput=False)
    bvec_d = nc.declare_dram_parameter("bvec", [116, 1], fp32,
                                       isOutput=False)
    h2_d = nc.declare_dram_parameter("h2", [nch, H, CHUNK * BL], f16,
                                     isOutput=True)

    ctx = ExitStack()
    sb = lambda shape, name, dt=fp32: ctx.enter_context(
        nc.sbuf_tensor(name, shape, dt))
    ps = lambda shape, name: ctx.enter_context(
        nc.psum_tensor(name, shape, fp32))
    sem = lambda name: ctx.enter_context(nc.semaphore(name))

    wh_raw = sb([SR, 116], "wh_raw", f16)
    bv_raw = sb([116, 1], "bv_raw")
    w_h = sb([SR, 116], "w_h", f16)   # rows 0:20 = W (h side), row 32 = w_x
    bv = sb([116, 1], "bv")
    b_hn2 = sb([52, 1], "b_hn2")   # b_hh n-gate at rows 32:52 (matches r)
    b_in2 = sb([20, 1], "b_in2")   # b_ih n-gate at rows 0:20 (matches tt)
    # shared [., BL] tiles; stream s uses cols s*HB:(s+1)*HB
    srz = sb([52, BL], "srz")
    tt = sb([20, BL], "tt")
    sn = sb([20, BL], "sn", f16)
    omz = sb([20, BL], "omz", f16)
    uu = sb([20, BL], "uu", f16)
    vv = sb([20, BL], "vv", f16)
    state = [sb([SR, CHUNK * BL], f"state{i}", f16) for i in range(3)]
    # P banks per stream (accumulation groups are per-bank); narg shared
    Pb = [[ps([116, _W[s]], f"P{s}{i}") for i in range(2)]
          for s in range(NS)]
    nargb = ps([20, BL], "narg")

    def Pv(s, i, rows=slice(0, 116)):
        return Pb[s][i][rows, :]

    def nargv(s):
        return nargb[:, _C0[s]:_C0[s] + _W[s]]

    sem_d = [sem(f"sem_d{s}") for s in range(NS)]  # DVE: memset + narg
    sem_a = [sem(f"sem_a{s}") for s in range(NS)]  # ACT: 2/step
    sem_p = [sem(f"sem_p{s}") for s in range(NS)]  # PE: P(k) complete
    sem_g = [sem(f"sem_g{s}") for s in range(NS)]  # Pool: uu, vv, h'
    dma_w = sem("dma_w")
    dma_x = [sem(f"dma_x{i}") for i in range(3)]  # x chunks, by c%3
    dma_o = [sem(f"dma_o{i}") for i in range(3)]  # h2 out, by c%3

    def slot_ap(s, strm, rows=slice(0, SR)):
        c0 = (s % CHUNK) * BL + _C0[strm]
        return state[(s // CHUNK) % 3][rows, c0:c0 + _W[strm]]

    def half(t, strm, rows=None):
        sl = slice(_C0[strm], _C0[strm] + _W[strm])
        if rows is None:
            return t[:, sl]
        return t[rows, sl]

    with nc.Block() as block:

        @block.sync
        def _(sp):
            sp.dma_start(wh_raw[:, :], wrec_d[:]).then_inc(dma_w, 16)
            sp.dma_start(bv_raw[:, :], bvec_d[:]).then_inc(dma_w, 16)
            # x for chunks 0..2 straight into state row XROW
            for c in range(min(3, nch)):
                sp.dma_start(state[c][XROW:XROW + 1, :], x_d[c]).then_inc(
                    dma_x[c], 16)
            for c in range(nch):
                # stream out chunk c once its last h' lands (both streams)
                last_k = min(CHUNK * c + CHUNK - 2, nstep - 1)
                for s in range(NS):
                    sp.wait_ge(sem_g[s], 3 * last_k + 3)
                sp.dma_start(h2_d[c], state[c % 3][H:2 * H, :]).then_inc(
                    dma_o[c % 3], 16)
                # refill x row of tile (c+3) once chunk c's matmuls done
                if c + 3 < nch:
                    for s in range(NS):
                        sp.wait_ge(sem_p[s], CHUNK * (c + 1))
                    sp.dma_start(state[c % 3][XROW:XROW + 1, :],
                                 x_d[c + 3]).then_inc(dma_x[c % 3], 16)

        @block.tensor
        def _(pe):
            # iter k builds P_s(k) = w_x@x(k) + W@uu(k-1) + W@vv(k-1)
            for k in range(nstep):
                if k % CHUNK == 0:
                    c = k // CHUNK
                    pe.wait_ge(dma_x[c % 3], 16 * (c // 3 + 1))
                for s in range(NS):
                    pe.wait_ge(sem_d[s], max(1, k))
                    mm = nc.tensor.matmul(
                        Pv(s, k % 2), w_h[XROW:XROW + 1, :],
                        slot_ap(k, s, slice(XROW, XROW + 1)),
                        start=True, stop=(k == 0))
                    if k == 0:
                        mm.then_inc(sem_p[s])
                        continue
                    pe.wait_ge(sem_g[s], 3 * (k - 1) + 1)
                    nc.tensor.matmul(Pv(s, k % 2), w_h[0:2 * H, :],
                                     half(uu, s), start=False, stop=False)
                    pe.wait_ge(sem_g[s], 3 * (k - 1) + 2)
                    nc.tensor.matmul(Pv(s, k % 2), w_h[0:2 * H, :],
                                     half(vv, s), start=False,
                                     stop=True).then_inc(sem_p[s])

        @block.scalar
        def _(act):
            for k in range(nstep):
                for s in range(NS):
                    act.wait_ge(sem_p[s], k + 1)
                    nc.scalar.activation(half(srz, s),
                                         Pv(s, k % 2, slice(0, 52)),
                                         Act.Sigmoid,
                                         bias=bv[0:52, :]).then_inc(sem_a[s])
                for s in range(NS):
                    act.wait_ge(sem_d[s], k + 2)
                    nc.scalar.activation(half(sn, s), nargv(s),
                                         Act.Tanh).then_inc(sem_a[s])

        @block.vector
        def _(dve):
            dve.wait_ge(dma_w, 32)
            nc.vector.tensor_copy(w_h[:, :], wh_raw[:, :])
            nc.vector.tensor_copy(bv[:, :], bv_raw[:, :])
            nc.vector.tensor_copy(b_hn2[32:52, :], bv_raw[64:84, :])
            nc.vector.tensor_copy(b_in2[:, :], bv_raw[96:116, :])
            # h(0) = 0 (slot 0 only; h'(0) full-writes slot 1)
            for s in range(NS):
                nc.vector.memset(
                    state[0][0:2 * H, _C0[s]:_C0[s] + _W[s]],
                    0.0).then_inc(sem_d[s])
            for k in range(nstep):
                for s in range(NS):
                    dve.wait_ge(sem_a[s], 2 * k + 1)
                    # t = (hn + b_hn) * r      (r @ rows 32:52)
                    nc.vector.scalar_tensor_tensor(
                        half(tt, s), Pv(s, k % 2, slice(64, 84)),
                        b_hn2[32:52, :], half(srz, s, slice(32, 52)),
                        op0=Alu.add, op1=Alu.mult)
                    # n_arg = (xn + b_in) + t
                    nc.vector.scalar_tensor_tensor(
                        nargv(s), Pv(s, k % 2, slice(96, 116)), b_in2[:, :],
                        half(tt, s), op0=Alu.add,
                        op1=Alu.add).then_inc(sem_d[s])

        @block.gpsimd
        def _(gp):
            # vv rows 10:20 must read as 0 at k=0 (layer-1 not live yet)
            nc.gpsimd.memset(vv[:, :], 0.0)
            for k in range(nstep):
                for s in range(NS):
                    gp.wait_ge(sem_a[s], 2 * k + 1)
                    # omz = 1 - z ; u = h * z   (z @ rows 0:20)
                    nc.gpsimd.tensor_scalar(half(omz, s),
                                            half(srz, s, slice(0, 20)),
                                            -1.0, 1.0,
                                            op0=Alu.mult, op1=Alu.add)
                    nc.gpsimd.tensor_mul(
                        half(uu, s), slot_ap(k, s, slice(0, 2 * H)),
                        half(srz, s, slice(0, 20))).then_inc(sem_g[s])
                for s in range(NS):
                    gp.wait_ge(sem_a[s], 2 * k + 2)
                    # vv = n * (1 - z)
                    if k == 0:
                        nc.gpsimd.tensor_mul(
                            half(vv, s, slice(0, H)),
                            half(sn, s, slice(0, H)),
                            half(omz, s, slice(0, H))).then_inc(sem_g[s])
                    else:
                        nc.gpsimd.tensor_mul(
                            half(vv, s), half(sn, s),
                            half(omz, s)).then_inc(sem_g[s])
                    # h' = vv + uu -> state slot k+1 (off critical path)
                    if s == 0 and (k + 1) % CHUNK == 0 and \
                            (k + 1) // CHUNK >= 3:
                        cp = (k + 1) // CHUNK
                        gp.wait_ge(dma_o[cp % 3], 16 * (cp // 3))
                    nc.gpsimd.tensor_add(
                        slot_ap(k + 1, s, slice(0, 2 * H)),
                        half(vv, s), half(uu, s)).then_inc(sem_g[s])

    return nc, nch


def _get_program(t_steps):
    if t_steps not in _PROGRAM_CACHE:
        _PROGRAM_CACHE[t_steps] = _build_program(t_steps)
    return _PROGRAM_CACHE[t_steps]


def _pack_weights(w_ih0, w_hh0, b_ih0, b_hh0, w_ih1, w_hh1, b_ih1, b_hh1):
    """Column layout (116): z0@0 z1@10 r0@32 r1@42 hn0@64 hn1@74
    xn0@96 xn1@106; rows = [h0(10); h1(10); pad; x@32]."""
    w_rec = np.zeros((SR, 116), np.float32)
    # rows 0:10 contract with h0
    w_rec[0:10, 32:42] = w_hh0[0:10, :].T      # r0
    w_rec[0:10, 0:10] = w_hh0[10:20, :].T      # z0
    w_rec[0:10, 64:74] = w_hh0[20:30, :].T     # hn0
    w_rec[0:10, 42:52] = w_ih1[0:10, :].T      # r1 (layer1 input side)
    w_rec[0:10, 10:20] = w_ih1[10:20, :].T     # z1
    w_rec[0:10, 106:116] = w_ih1[20:30, :].T   # xn1
    # rows 10:20 contract with h1
    w_rec[10:20, 42:52] = w_hh1[0:10, :].T     # r1
    w_rec[10:20, 10:20] = w_hh1[10:20, :].T    # z1
    w_rec[10:20, 74:84] = w_hh1[20:30, :].T    # hn1
    # row XROW contracts with x_t (rank-1 layer0 input side)
    w_rec[XROW, 32:42] = w_ih0[0:10, 0]          # r0
    w_rec[XROW, 0:10] = w_ih0[10:20, 0]          # z0
    w_rec[XROW, 96:106] = w_ih0[20:30, 0]        # xn0

    bvec = np.zeros((116, 1), np.float32)
    bvec[32:42, 0] = b_ih0[0:10] + b_hh0[0:10]     # r0
    bvec[42:52, 0] = b_ih1[0:10] + b_hh1[0:10]     # r1
    bvec[0:10, 0] = b_ih0[10:20] + b_hh0[10:20]    # z0
    bvec[10:20, 0] = b_ih1[10:20] + b_hh1[10:20]   # z1
    bvec[64:74, 0] = b_hh0[20:30]                  # hn0 (inside r*)
    bvec[74:84, 0] = b_hh1[20:30]                  # hn1
    bvec[96:106, 0] = b_ih0[20:30]                 # xn0
    bvec[106:116, 0] = b_ih1[20:30]                # xn1
    return w_rec.astype(F16), bvec


OUTPUT_NAMES = ["h2"]


def _make_in_maps(x, inputs, t_steps):
    nstep, nslot, nch = _dims(t_steps)
    w_rec, bvec = _pack_weights(
        np.asarray(inputs["w_ih0"], np.float32),
        np.asarray(inputs["w_hh0"], np.float32),
        np.asarray(inputs["b_ih0"], np.float32),
        np.asarray(inputs["b_hh0"], np.float32),
        np.asarray(inputs["w_ih1"], np.float32),
        np.asarray(inputs["w_hh1"], np.float32),
        np.asarray(inputs["b_ih1"], np.float32),
        np.asarray(inputs["b_hh1"], np.float32))
    in_maps = []
    for c in range(NCORES):
        xc = x[c * BL:(c + 1) * BL, :, 0]          # [BL, t]
        xt = np.zeros((nch * CHUNK, BL), np.float32)
        xt[:t_steps, :] = xc.T
        in_maps.append({
            "xt": xt.reshape(nch, CHUNK * BL).astype(F16),
            "w_rec": w_rec, "bvec": bvec,
        })
    return in_maps


def _postprocess_core(outs, inputs, t_steps):
    """outs: dict of this core's output tensors -> [BL, t, 1] block."""
    _, _, nch = _dims(t_steps)
    w_lin = np.asarray(inputs["w_lin"], np.float32)
    b_lin = np.asarray(inputs["b_lin"], np.float32)
    h2 = np.asarray(outs["h2"], np.float32)    # [nch, H, CHUNK*BL]
    # slot s holds layer-1 state after macro-step s-1, which processed
    # layer-1 timestep t' = s-2  =>  h2[b, t] = slot t+2
    arr = h2.reshape(nch, H, CHUNK, BL)
    arr = arr.transpose(3, 0, 2, 1).reshape(BL, nch * CHUNK, H)
    h2_bt = arr[:, 2:t_steps + 2, :]           # [BL, t, H]
    out = np.empty((BL, t_steps, 1), np.float32)
    out[:, :, 0] = h2_bt @ w_lin[0, :] + b_lin[0]
    return out


def run(x, w_ih0, w_hh0, b_ih0, b_hh0, w_ih1, w_hh1, b_ih1, b_hh1,
        w_lin, b_lin, t_steps=T, trace=False):
    from concourse.bass_utils import run_bass_kernel_spmd

    nc, nch = _get_program(t_steps)

    x = np.asarray(x, np.float32)
    inputs = dict(w_ih0=w_ih0, w_hh0=w_hh0, b_ih0=b_ih0, b_hh0=b_hh0,
                  w_ih1=w_ih1, w_hh1=w_hh1, b_ih1=b_ih1, b_hh1=b_hh1,
                  w_lin=w_lin, b_lin=b_lin)
    in_maps = _make_in_maps(x, inputs, t_steps)

    res = run_bass_kernel_spmd(nc, in_maps, list(range(NCORES)), trace=trace)

    out = np.empty((B, t_steps, 1), np.float32)
    for c in range(NCORES):
        out[c * BL:(c + 1) * BL] = _postprocess_core(
            res.results[c], inputs, t_steps)
    return out, res


def kernel(x, w_ih0, w_hh0, b_ih0, b_hh0, w_ih1, w_hh1, b_ih1, b_hh1,
           w_lin, b_lin):
    out, _ = run(x, w_ih0, w_hh0, b_ih0, b_hh0, w_ih1, w_hh1, b_ih1, b_hh1,
                 w_lin, b_lin)
    return out


# revision 26
# speedup vs baseline: 1.0161x; 1.0161x over previous
"""Trainium2 Bass kernel for a 2-layer GRU (H=10) + linear head.

Strategy (pure data parallel, 8 cores):
  - Shard batch B=1024 -> 128 per core; replicate the tiny weights.
  - Per core, the T=2048 recurrence runs as one fused chain covering BOTH
    GRU layers (layer 1 lags layer 0 by one step, software-pipelined).
  - fp16 state/weights: PE runs at 1 cycle/row (fp32 is 4).
  - PSUM-accumulated recurrence: h' = vv + uu  (vv = n*(1-z), uu = h*z,
    both on Pool), and W @ h' = W@vv + W@uu, so the step-(k+1)
    preactivations are built by THREE accumulating matmuls into one PSUM
    bank: mm_x (rank-1 x side, start=True), mm_uu, mm_vv.  The h'
    materialization (Pool add into the state chunk tile) is OFF the
    critical path - the next matmul fires as soon as vv lands.
  - TWO staggered streams per core (batch cols 0:64 / 64:128).  Each
    stream has its own PSUM banks + semaphores and runs its latency
    chain independently; engines interleave A/B instructions, and the
    narrower (64-col) instructions shorten the chain itself.
  - hb = hn + b_hn is prestaged on DVE while sigma runs (gated only on
    the matmul), so the post-sigma DVE work is tt = hb*r (all-SBUF)
    then narg = (xn + b_in) + tt.  Steady-state chain ~1.42us/step:
    sigma -> tt,narg (DVE) -> tanh -> vv (Pool) -> mm_vv (PE) ->
    sigma, each hop paying the 100ns semaphore delay.
  - PSUM layout per stream ([116, 64] fp32): z 0:20 | r 32:52 |
    hn 64:84 | xn 96:116 (20-row blocks at 32-row quadrant bases);
    narg in a shared [20, 128] bank.  NOTE: hardware rejects PSUM
    accumulation groups that interleave with other banks' groups and
    engine ops whose two SBUF inputs have different base partitions -
    CoreSim checks neither.
  - h' states accumulate in [33, CHUNK*128] fp16 chunk tiles (rows 0:20
    h0|h1, row 32 = x_t DMA'd in); layer-1 rows DMA out per chunk; the
    final linear head (10 -> 1) runs on host.
"""

import numpy as np
F16 = np.float16

H = 10
B = 1024
T = 2048
NCORES = 8
BL = B // NCORES  # 128 batch rows per core
NS = 3            # staggered streams per core
_W = [BL // NS + (1 if i < BL % NS else 0) for i in range(NS)]
_C0 = [sum(_W[:i]) for i in range(NS)]    # col offset of stream i
CHUNK = 64        # time slots per SBUF state chunk
SR = 33           # state rows: h0(10) | h1(10) | pad | x @ 32
XROW = 32         # x row partition (matmul base partition must be 0/32/64)

_PROGRAM_CACHE = {}


def _dims(t_steps):
    nstep = t_steps + 1          # macro-steps 0..t (layer1 lags by one)
    nslot = nstep + 1            # state slots
    nch = (nslot + CHUNK - 1) // CHUNK
    return nstep, nslot, nch


def _build_program(t_steps):
    from contextlib import ExitStack

    import concourse.bass as bass
    import concourse.mybir as mybir

    fp32 = mybir.dt.float32
    f16 = mybir.dt.float16
    Alu = mybir.AluOpType
    Act = mybir.ActivationFunctionType

    nstep, nslot, nch = _dims(t_steps)

    nc = bass.Bass()

    x_d = nc.declare_dram_parameter("xt", [nch, CHUNK * BL], f16,
                                    isOutput=False)
    wrec_d = nc.declare_dram_parameter("w_rec", [SR, 116], f16,
                                       isOutput=False)
    bvec_d = nc.declare_dram_parameter("bvec", [116, 1], fp32,
                                       isOutput=False)
    h2_d = nc.declare_dram_parameter("h2", [nch, H, CHUNK * BL], f16,
                                     isOutput=True)

    ctx = ExitStack()
    sb = lambda shape, name, dt=fp32: ctx.enter_context(
        nc.sbuf_tensor(name, shape, dt))
    ps = lambda shape, name: ctx.enter_context(
        nc.psum_tensor(name, shape, fp32))
    sem = lambda name: ctx.enter_context(nc.semaphore(name))

    wh_raw = sb([SR, 116], "wh_raw", f16)
    bv_raw = sb([116, 1], "bv_raw")
    w_h = sb([SR, 116], "w_h", f16)   # rows 0:20 = W (h side), row 32 = w_x
    bv = sb([116, 1], "bv")
    b_hn2 = sb([52, 1], "b_hn2")   # b_hh n-gate at rows 32:52 (matches r)
    b_in2 = sb([20, 1], "b_in2")   # b_ih n-gate at rows 0:20 (matches tt)
    # shared [., BL] tiles; stream s uses cols s*HB:(s+1)*HB
    srz = sb([52, BL], "srz")
    hb = sb([52, BL], "hb")        # hn + b_hn staged at rows 32:52
    tt = sb([20, BL], "tt")
    sn = sb([20, BL], "sn", f16)
    omz = sb([20, BL], "omz", f16)
    uu = sb([20, BL], "uu", f16)
    vv = sb([20, BL], "vv", f16)
    state = [sb([SR, CHUNK * BL], f"state{i}", f16) for i in range(3)]
    # P banks per stream (accumulation groups are per-bank); narg shared
    Pb = [[ps([116, _W[s]], f"P{s}{i}") for i in range(2)]
          for s in range(NS)]
    nargb = ps([20, BL], "narg")

    def Pv(s, i, rows=slice(0, 116)):
        return Pb[s][i][rows, :]

    def nargv(s):
        return nargb[:, _C0[s]:_C0[s] + _W[s]]

    sem_d = [sem(f"sem_d{s}") for s in range(NS)]  # DVE: memset + narg
    sem_a = [sem(f"sem_a{s}") for s in range(NS)]  # ACT: 2/step
    sem_p = [sem(f"sem_p{s}") for s in range(NS)]  # PE: P(k) complete
    sem_g = [sem(f"sem_g{s}") for s in range(NS)]  # Pool: uu, vv, h'
    dma_w = sem("dma_w")
    dma_x = [sem(f"dma_x{i}") for i in range(3)]  # x chunks, by c%3
    dma_o = [sem(f"dma_o{i}") for i in range(3)]  # h2 out, by c%3

    def slot_ap(s, strm, rows=slice(0, SR)):
        c0 = (s % CHUNK) * BL + _C0[strm]
        return state[(s // CHUNK) % 3][rows, c0:c0 + _W[strm]]

    def half(t, strm, rows=None):
        sl = slice(_C0[strm], _C0[strm] + _W[strm])
        if rows is None:
            return t[:, sl]
        return t[rows, sl]

    with nc.Block() as block:

        @block.sync
        def _(sp):
            sp.dma_start(wh_raw[:, :], wrec_d[:]).then_inc(dma_w, 16)
            sp.dma_start(bv_raw[:, :], bvec_d[:]).then_inc(dma_w, 16)
            # x for chunks 0..2 straight into state row XROW
            for c in range(min(3, nch)):
                sp.dma_start(state[c][XROW:XROW + 1, :], x_d[c]).then_inc(
                    dma_x[c], 16)
            for c in range(nch):
                # stream out chunk c once its last h' lands (both streams)
                last_k = min(CHUNK * c + CHUNK - 2, nstep - 1)
                for s in range(NS):
                    sp.wait_ge(sem_g[s], 3 * last_k + 3)
                sp.dma_start(h2_d[c], state[c % 3][H:2 * H, :]).then_inc(
                    dma_o[c % 3], 16)
                # refill x row of tile (c+3) once chunk c's matmuls done
                if c + 3 < nch:
                    for s in range(NS):
                        sp.wait_ge(sem_p[s], CHUNK * (c + 1))
                    sp.dma_start(state[c % 3][XROW:XROW + 1, :],
                                 x_d[c + 3]).then_inc(dma_x[c % 3], 16)

        @block.tensor
        def _(pe):
            # iter k builds P_s(k) = w_x@x(k) + W@uu(k-1) + W@vv(k-1)
            for k in range(nstep):
                if k % CHUNK == 0:
                    c = k // CHUNK
                    pe.wait_ge(dma_x[c % 3], 16 * (c // 3 + 1))
                for s in range(NS):
                    pe.wait_ge(sem_d[s], max(1, k))
                    mm = nc.tensor.matmul(
                        Pv(s, k % 2), w_h[XROW:XROW + 1, :],
                        slot_ap(k, s, slice(XROW, XROW + 1)),
                        start=True, stop=(k == 0))
                    if k == 0:
                        mm.then_inc(sem_p[s])
                        continue
                    pe.wait_ge(sem_g[s], 3 * (k - 1) + 1)
                    nc.tensor.matmul(Pv(s, k % 2), w_h[0:2 * H, :],
                                     half(uu, s), start=False, stop=False)
                    pe.wait_ge(sem_g[s], 3 * (k - 1) + 2)
                    nc.tensor.matmul(Pv(s, k % 2), w_h[0:2 * H, :],
                                     half(vv, s), start=False,
                                     stop=True).then_inc(sem_p[s])

        @block.scalar
        def _(act):
            for k in range(nstep):
                for s in range(NS):
                    act.wait_ge(sem_p[s], k + 1)
                    nc.scalar.activation(half(srz, s),
                                         Pv(s, k % 2, slice(0, 52)),
                                         Act.Sigmoid,
                                         bias=bv[0:52, :]).then_inc(sem_a[s])
                for s in range(NS):
                    act.wait_ge(sem_d[s], k + 2)
                    nc.scalar.activation(half(sn, s), nargv(s),
                                         Act.Tanh).then_inc(sem_a[s])

        @block.vector
        def _(dve):
            dve.wait_ge(dma_w, 32)
            nc.vector.tensor_copy(w_h[:, :], wh_raw[:, :])
            nc.vector.tensor_copy(bv[:, :], bv_raw[:, :])
            nc.vector.tensor_copy(b_hn2[32:52, :], bv_raw[64:84, :])
            nc.vector.tensor_copy(b_in2[:, :], bv_raw[96:116, :])
            # h(0) = 0 (slot 0 only; h'(0) full-writes slot 1)
            for s in range(NS):
                nc.vector.memset(
                    state[0][0:2 * H, _C0[s]:_C0[s] + _W[s]],
                    0.0).then_inc(sem_d[s])
            for s in range(NS):
                dve.wait_ge(sem_p[s], 1)
                nc.vector.tensor_scalar_add(
                    half(hb, s, slice(32, 52)),
                    Pv(s, 0, slice(64, 84)), b_hn2[32:52, :])
            for k in range(nstep):
                for s in range(NS):
                    dve.wait_ge(sem_a[s], 2 * k + 1)
                    # t = hb * r  (hb prestaged last iter; both SBUF @32:52)
                    nc.vector.tensor_mul(
                        half(tt, s), half(hb, s, slice(32, 52)),
                        half(srz, s, slice(32, 52)))
                    # n_arg = (xn + b_in) + t
                    nc.vector.scalar_tensor_tensor(
                        nargv(s), Pv(s, k % 2, slice(96, 116)), b_in2[:, :],
                        half(tt, s), op0=Alu.add,
                        op1=Alu.add).then_inc(sem_d[s])
                # prestage hb(k+1) = hn + b_hn while sigma(k+1) runs
                if k + 1 < nstep:
                    for s in range(NS):
                        dve.wait_ge(sem_p[s], k + 2)
                        nc.vector.tensor_scalar_add(
                            half(hb, s, slice(32, 52)),
                            Pv(s, (k + 1) % 2, slice(64, 84)),
                            b_hn2[32:52, :])

        @block.gpsimd
        def _(gp):
            # vv rows 10:20 must read as 0 at k=0 (layer-1 not live yet)
            nc.gpsimd.memset(vv[:, :], 0.0)
            for k in range(nstep):
                for s in range(NS):
                    gp.wait_ge(sem_a[s], 2 * k + 1)
                    # omz = 1 - z ; u = h * z   (z @ rows 0:20)
                    nc.gpsimd.tensor_scalar(half(omz, s),
                                            half(srz, s, slice(0, 20)),
                                            -1.0, 1.0,
                                            op0=Alu.mult, op1=Alu.add)
                    nc.gpsimd.tensor_mul(
                        half(uu, s), slot_ap(k, s, slice(0, 2 * H)),
                        half(srz, s, slice(0, 20))).then_inc(sem_g[s])
                for s in range(NS):
                    gp.wait_ge(sem_a[s], 2 * k + 2)
                    # vv = n * (1 - z)
                    if k == 0:
                        nc.gpsimd.tensor_mul(
                            half(vv, s, slice(0, H)),
                            half(sn, s, slice(0, H)),
                            half(omz, s, slice(0, H))).then_inc(sem_g[s])
                    else:
                        nc.gpsimd.tensor_mul(
                            half(vv, s), half(sn, s),
                            half(omz, s)).then_inc(sem_g[s])
                    # h' = vv + uu -> state slot k+1 (off critical path)
                    if s == 0 and (k + 1) % CHUNK == 0 and \
                            (k + 1) // CHUNK >= 3:
                        cp = (k + 1) // CHUNK
                        gp.wait_ge(dma_o[cp % 3], 16 * (cp // 3))
                    nc.gpsimd.tensor_add(
                        slot_ap(k + 1, s, slice(0, 2 * H)),
                        half(vv, s), half(uu, s)).then_inc(sem_g[s])

    return nc, nch


def _get_program(t_steps):
    if t_steps not in _PROGRAM_CACHE:
        _PROGRAM_CACHE[t_steps] = _build_program(t_steps)
    return _PROGRAM_CACHE[t_steps]


def _pack_weights(w_ih0, w_hh0, b_ih0, b_hh0, w_ih1, w_hh1, b_ih1, b_hh1):
    """Column layout (116): z0@0 z1@10 r0@32 r1@42 hn0@64 hn1@74
    xn0@96 xn1@106; rows = [h0(10); h1(10); pad; x@32]."""
    w_rec = np.zeros((SR, 116), np.float32)
    # rows 0:10 contract with h0
    w_rec[0:10, 32:42] = w_hh0[0:10, :].T      # r0
    w_rec[0:10, 0:10] = w_hh0[10:20, :].T      # z0
    w_rec[0:10, 64:74] = w_hh0[20:30, :].T     # hn0
    w_rec[0:10, 42:52] = w_ih1[0:10, :].T      # r1 (layer1 input side)
    w_rec[0:10, 10:20] = w_ih1[10:20, :].T     # z1
    w_rec[0:10, 106:116] = w_ih1[20:30, :].T   # xn1
    # rows 10:20 contract with h1
    w_rec[10:20, 42:52] = w_hh1[0:10, :].T     # r1
    w_rec[10:20, 10:20] = w_hh1[10:20, :].T    # z1
    w_rec[10:20, 74:84] = w_hh1[20:30, :].T    # hn1
    # row XROW contracts with x_t (rank-1 layer0 input side)
    w_rec[XROW, 32:42] = w_ih0[0:10, 0]          # r0
    w_rec[XROW, 0:10] = w_ih0[10:20, 0]          # z0
    w_rec[XROW, 96:106] = w_ih0[20:30, 0]        # xn0

    bvec = np.zeros((116, 1), np.float32)
    bvec[32:42, 0] = b_ih0[0:10] + b_hh0[0:10]     # r0
    bvec[42:52, 0] = b_ih1[0:10] + b_hh1[0:10]     # r1
    bvec[0:10, 0] = b_ih0[10:20] + b_hh0[10:20]    # z0
    bvec[10:20, 0] = b_ih1[10:20] + b_hh1[10:20]   # z1
    bvec[64:74, 0] = b_hh0[20:30]                  # hn0 (inside r*)
    bvec[74:84, 0] = b_hh1[20:30]                  # hn1
    bvec[96:106, 0] = b_ih0[20:30]                 # xn0
    bvec[106:116, 0] = b_ih1[20:30]                # xn1
    return w_rec.astype(F16), bvec


OUTPUT_NAMES = ["h2"]


def _make_in_maps(x, inputs, t_steps):
    nstep, nslot, nch = _dims(t_steps)
    w_rec, bvec = _pack_weights(
        np.asarray(inputs["w_ih0"], np.float32),
        np.asarray(inputs["w_hh0"], np.float32),
        np.asarray(inputs["b_ih0"], np.float32),
        np.asarray(inputs["b_hh0"], np.float32),
        np.asarray(inputs["w_ih1"], np.float32),
        np.asarray(inputs["w_hh1"], np.float32),
        np.asarray(inputs["b_ih1"], np.float32),
        np.asarray(inputs["b_hh1"], np.float32))
    in_maps = []
    for c in range(NCORES):
        xc = x[c * BL:(c + 1) * BL, :, 0]          # [BL, t]
        xt = np.zeros((nch * CHUNK, BL), np.float32)
        xt[:t_steps, :] = xc.T
        in_maps.append({
            "xt": xt.reshape(nch, CHUNK * BL).astype(F16),
            "w_rec": w_rec, "bvec": bvec,
        })
    return in_maps


def _postprocess_core(outs, inputs, t_steps):
    """outs: dict of this core's output tensors -> [BL, t, 1] block."""
    _, _, nch = _dims(t_steps)
    w_lin = np.asarray(inputs["w_lin"], np.float32)
    b_lin = np.asarray(inputs["b_lin"], np.float32)
    h2 = np.asarray(outs["h2"], np.float32)    # [nch, H, CHUNK*BL]
    # slot s holds layer-1 state after macro-step s-1, which processed
    # layer-1 timestep t' = s-2  =>  h2[b, t] = slot t+2
    arr = h2.reshape(nch, H, CHUNK, BL)
    arr = arr.transpose(3, 0, 2, 1).reshape(BL, nch * CHUNK, H)
    h2_bt = arr[:, 2:t_steps + 2, :]           # [BL, t, H]
    out = np.empty((BL, t_steps, 1), np.float32)
    out[:, :, 0] = h2_bt @ w_lin[0, :] + b_lin[0]
    return out


def run(x, w_ih0, w_hh0, b_ih0, b_hh0, w_ih1, w_hh1, b_ih1, b_hh1,
        w_lin, b_lin, t_steps=T, trace=False):
    from concourse.bass_utils import run_bass_kernel_spmd

    nc, nch = _get_program(t_steps)

    x = np.asarray(x, np.float32)
    inputs = dict(w_ih0=w_ih0, w_hh0=w_hh0, b_ih0=b_ih0, b_hh0=b_hh0,
                  w_ih1=w_ih1, w_hh1=w_hh1, b_ih1=b_ih1, b_hh1=b_hh1,
                  w_lin=w_lin, b_lin=b_lin)
    in_maps = _make_in_maps(x, inputs, t_steps)

    res = run_bass_kernel_spmd(nc, in_maps, list(range(NCORES)), trace=trace)

    out = np.empty((B, t_steps, 1), np.float32)
    for c in range(NCORES):
        out[c * BL:(c + 1) * BL] = _postprocess_core(
            res.results[c], inputs, t_steps)
    return out, res


def kernel(x, w_ih0, w_hh0, b_ih0, b_hh0, w_ih1, w_hh1, b_ih1, b_hh1,
           w_lin, b_lin):
    out, _ = run(x, w_ih0, w_hh0, b_ih0, b_hh0, w_ih1, w_hh1, b_ih1, b_hh1,
                 w_lin, b_lin)
    return out


# revision 33
# speedup vs baseline: 1.0209x; 1.0047x over previous
"""Trainium2 Bass kernel for a 2-layer GRU (H=10) + linear head.

Strategy (pure data parallel, 8 cores):
  - Shard batch B=1024 -> 128 per core; replicate the tiny weights.
  - Per core, the T=2048 recurrence runs as one fused chain covering BOTH
    GRU layers (layer 1 lags layer 0 by one step, software-pipelined).
  - fp16 state/weights: PE runs at 1 cycle/row (fp32 is 4).
  - PSUM-accumulated recurrence: h' = vv + uu  (vv = n*(1-z), uu = h*z,
    both on Pool), and W @ h' = W@vv + W@uu, so the step-(k+1)
    preactivations are built by THREE accumulating matmuls into one PSUM
    bank: mm_x (rank-1 x side, start=True), mm_uu, mm_vv.  The h'
    materialization (Pool add into the state chunk tile) is OFF the
    critical path - the next matmul fires as soon as vv lands.
  - TWO staggered streams per core (batch cols 0:64 / 64:128).  Each
    stream has its own PSUM banks + semaphores and runs its latency
    chain independently; engines interleave A/B instructions, and the
    narrower (64-col) instructions shorten the chain itself.
  - hb = hn + b_hn is prestaged on DVE while sigma runs (gated only on
    the matmul), so the post-sigma DVE work is tt = hb*r (all-SBUF)
    then narg = (xn + b_in) + tt.  Steady-state chain ~1.42us/step:
    sigma -> tt,narg (DVE) -> tanh -> vv (Pool) -> mm_vv (PE) ->
    sigma, each hop paying the 100ns semaphore delay.
  - PSUM layout per stream ([116, 64] fp32): z 0:20 | r 32:52 |
    hn 64:84 | xn 96:116 (20-row blocks at 32-row quadrant bases);
    narg in a shared [20, 128] bank.  NOTE: hardware rejects PSUM
    accumulation groups that interleave with other banks' groups and
    engine ops whose two SBUF inputs have different base partitions -
    CoreSim checks neither.
  - h' states accumulate in [33, CHUNK*128] fp16 chunk tiles (rows 0:20
    h0|h1, row 32 = x_t DMA'd in); layer-1 rows DMA out per chunk; the
    final linear head (10 -> 1) runs on host.
"""

import numpy as np
F16 = np.float16

H = 10
B = 1024
T = 2048
NCORES = 8
BL = B // NCORES  # 128 batch rows per core
NS = 3            # staggered streams per core
_W = [BL // NS + (1 if i < BL % NS else 0) for i in range(NS)]
_C0 = [sum(_W[:i]) for i in range(NS)]    # col offset of stream i
CHUNK = 8         # time slots per SBUF state chunk
SR = 33           # state rows: h0(10) | h1(10) | pad | x @ 32
XROW = 32         # x row partition (matmul base partition must be 0/32/64)

_PROGRAM_CACHE = {}


def _dims(t_steps):
    nstep = t_steps + 1          # macro-steps 0..t (layer1 lags by one)
    nslot = nstep + 1            # state slots
    nch = (nslot + CHUNK - 1) // CHUNK
    return nstep, nslot, nch


def _build_program(t_steps):
    from contextlib import ExitStack

    import concourse.bass as bass
    import concourse.mybir as mybir

    fp32 = mybir.dt.float32
    f16 = mybir.dt.float16
    Alu = mybir.AluOpType
    Act = mybir.ActivationFunctionType

    nstep, nslot, nch = _dims(t_steps)

    nc = bass.Bass()

    x_d = nc.declare_dram_parameter("xt", [nch, CHUNK * BL], f16,
                                    isOutput=False)
    wrec_d = nc.declare_dram_parameter("w_rec", [SR, 116], f16,
                                       isOutput=False)
    bvec_d = nc.declare_dram_parameter("bvec", [116, 1], fp32,
                                       isOutput=False)
    h2_d = nc.declare_dram_parameter("h2", [nch, H, CHUNK * BL], f16,
                                     isOutput=True)

    ctx = ExitStack()
    sb = lambda shape, name, dt=fp32: ctx.enter_context(
        nc.sbuf_tensor(name, shape, dt))
    ps = lambda shape, name: ctx.enter_context(
        nc.psum_tensor(name, shape, fp32))
    sem = lambda name: ctx.enter_context(nc.semaphore(name))

    wh_raw = sb([SR, 116], "wh_raw", f16)
    bv_raw = sb([116, 1], "bv_raw")
    w_h = sb([SR, 116], "w_h", f16)   # rows 0:20 = W (h side), row 32 = w_x
    bv = sb([116, 1], "bv")
    b_hn2 = sb([52, 1], "b_hn2")   # b_hh n-gate at rows 32:52 (matches r)
    b_in2 = sb([20, 1], "b_in2")   # b_ih n-gate at rows 0:20 (matches tt)
    # shared [., BL] tiles; stream s uses cols s*HB:(s+1)*HB
    srz = sb([52, BL], "srz")
    hb = sb([52, BL], "hb")        # hn + b_hn staged at rows 32:52
    tt = sb([20, BL], "tt")
    sn = sb([20, BL], "sn", f16)
    omz = sb([20, BL], "omz", f16)
    uu = sb([20, BL], "uu", f16)
    vv = sb([20, BL], "vv", f16)
    state = [sb([SR, CHUNK * BL], f"state{i}", f16) for i in range(3)]
    # P banks per stream (accumulation groups are per-bank); narg shared
    Pb = [[ps([116, _W[s]], f"P{s}{i}") for i in range(2)]
          for s in range(NS)]
    nargb = ps([20, BL], "narg")

    def Pv(s, i, rows=slice(0, 116)):
        return Pb[s][i][rows, :]

    def nargv(s):
        return nargb[:, _C0[s]:_C0[s] + _W[s]]

    sem_d = [sem(f"sem_d{s}") for s in range(NS)]  # DVE: memset + narg
    sem_a = [sem(f"sem_a{s}") for s in range(NS)]  # ACT: 2/step
    sem_p = [sem(f"sem_p{s}") for s in range(NS)]  # PE: P(k) complete
    sem_g = [sem(f"sem_g{s}") for s in range(NS)]  # Pool: uu, vv, h'
    dma_w = sem("dma_w")
    dma_x = [sem(f"dma_x{i}") for i in range(3)]  # x chunks, by c%3
    dma_o = [sem(f"dma_o{i}") for i in range(3)]  # h2 out, by c%3

    def slot_ap(s, strm, rows=slice(0, SR)):
        c0 = (s % CHUNK) * BL + _C0[strm]
        return state[(s // CHUNK) % 3][rows, c0:c0 + _W[strm]]

    def half(t, strm, rows=None):
        sl = slice(_C0[strm], _C0[strm] + _W[strm])
        if rows is None:
            return t[:, sl]
        return t[rows, sl]

    with nc.Block() as block:

        @block.sync
        def _(sp):
            sp.dma_start(wh_raw[:, :], wrec_d[:]).then_inc(dma_w, 16)
            sp.dma_start(bv_raw[:, :], bvec_d[:]).then_inc(dma_w, 16)
            # x for chunks 0..2 straight into state row XROW
            for c in range(min(3, nch)):
                sp.dma_start(state[c][XROW:XROW + 1, :], x_d[c]).then_inc(
                    dma_x[c], 16)
            for c in range(nch):
                # stream out chunk c once its last h' lands (both streams)
                last_k = min(CHUNK * c + CHUNK - 2, nstep - 1)
                for s in range(NS):
                    sp.wait_ge(sem_g[s], 3 * last_k + 3)
                sp.dma_start(h2_d[c], state[c % 3][H:2 * H, :]).then_inc(
                    dma_o[c % 3], 16)
                # refill x row of tile (c+3) once chunk c's matmuls done
                if c + 3 < nch:
                    for s in range(NS):
                        sp.wait_ge(sem_p[s], CHUNK * (c + 1))
                    sp.dma_start(state[c % 3][XROW:XROW + 1, :],
                                 x_d[c + 3]).then_inc(dma_x[c % 3], 16)

        @block.tensor
        def _(pe):
            # iter k builds P_s(k) = w_x@x(k) + W@uu(k-1) + W@vv(k-1)
            for k in range(nstep):
                if k % CHUNK == 0:
                    c = k // CHUNK
                    pe.wait_ge(dma_x[c % 3], 16 * (c // 3 + 1))
                for s in range(NS):
                    pe.wait_ge(sem_d[s], max(1, k))
                    mm = nc.tensor.matmul(
                        Pv(s, k % 2), w_h[XROW:XROW + 1, :],
                        slot_ap(k, s, slice(XROW, XROW + 1)),
                        start=True, stop=(k == 0))
                    if k == 0:
                        mm.then_inc(sem_p[s])
                        continue
                    pe.wait_ge(sem_g[s], 3 * (k - 1) + 1)
                    nc.tensor.matmul(Pv(s, k % 2), w_h[0:2 * H, :],
                                     half(uu, s), start=False, stop=False)
                    pe.wait_ge(sem_g[s], 3 * (k - 1) + 2)
                    nc.tensor.matmul(Pv(s, k % 2), w_h[0:2 * H, :],
                                     half(vv, s), start=False,
                                     stop=True).then_inc(sem_p[s])

        @block.scalar
        def _(act):
            for k in range(nstep):
                for s in range(NS):
                    act.wait_ge(sem_p[s], k + 1)
                    nc.scalar.activation(half(srz, s),
                                         Pv(s, k % 2, slice(0, 52)),
                                         Act.Sigmoid,
                                         bias=bv[0:52, :]).then_inc(sem_a[s])
                for s in range(NS):
                    act.wait_ge(sem_d[s], k + 2)
                    nc.scalar.activation(half(sn, s), nargv(s),
                                         Act.Tanh).then_inc(sem_a[s])

        @block.vector
        def _(dve):
            dve.wait_ge(dma_w, 32)
            nc.vector.tensor_copy(w_h[:, :], wh_raw[:, :])
            nc.vector.tensor_copy(bv[:, :], bv_raw[:, :])
            nc.vector.tensor_copy(b_hn2[32:52, :], bv_raw[64:84, :])
            nc.vector.tensor_copy(b_in2[:, :], bv_raw[96:116, :])
            # h(0) = 0 (slot 0 only; h'(0) full-writes slot 1)
            for s in range(NS):
                nc.vector.memset(
                    state[0][0:2 * H, _C0[s]:_C0[s] + _W[s]],
                    0.0).then_inc(sem_d[s])
            for s in range(NS):
                dve.wait_ge(sem_p[s], 1)
                nc.vector.tensor_scalar_add(
                    half(hb, s, slice(32, 52)),
                    Pv(s, 0, slice(64, 84)), b_hn2[32:52, :])
            for k in range(nstep):
                for s in range(NS):
                    dve.wait_ge(sem_a[s], 2 * k + 1)
                    # t = hb * r  (hb prestaged last iter; both SBUF @32:52)
                    nc.vector.tensor_mul(
                        half(tt, s), half(hb, s, slice(32, 52)),
                        half(srz, s, slice(32, 52)))
                    # n_arg = (xn + b_in) + t
                    nc.vector.scalar_tensor_tensor(
                        nargv(s), Pv(s, k % 2, slice(96, 116)), b_in2[:, :],
                        half(tt, s), op0=Alu.add,
                        op1=Alu.add).then_inc(sem_d[s])
                # prestage hb(k+1) = hn + b_hn while sigma(k+1) runs
                if k + 1 < nstep:
                    for s in range(NS):
                        dve.wait_ge(sem_p[s], k + 2)
                        nc.vector.tensor_scalar_add(
                            half(hb, s, slice(32, 52)),
                            Pv(s, (k + 1) % 2, slice(64, 84)),
                            b_hn2[32:52, :])

        @block.gpsimd
        def _(gp):
            # vv rows 10:20 must read as 0 at k=0 (layer-1 not live yet)
            nc.gpsimd.memset(vv[:, :], 0.0)
            for k in range(nstep):
                for s in range(NS):
                    gp.wait_ge(sem_a[s], 2 * k + 1)
                    # omz = 1 - z ; u = h * z   (z @ rows 0:20)
                    nc.gpsimd.tensor_scalar(half(omz, s),
                                            half(srz, s, slice(0, 20)),
                                            -1.0, 1.0,
                                            op0=Alu.mult, op1=Alu.add)
                    nc.gpsimd.tensor_mul(
                        half(uu, s), slot_ap(k, s, slice(0, 2 * H)),
                        half(srz, s, slice(0, 20))).then_inc(sem_g[s])
                for s in range(NS):
                    gp.wait_ge(sem_a[s], 2 * k + 2)
                    # vv = n * (1 - z)
                    if k == 0:
                        nc.gpsimd.tensor_mul(
                            half(vv, s, slice(0, H)),
                            half(sn, s, slice(0, H)),
                            half(omz, s, slice(0, H))).then_inc(sem_g[s])
                    else:
                        nc.gpsimd.tensor_mul(
                            half(vv, s), half(sn, s),
                            half(omz, s)).then_inc(sem_g[s])
                    # h' = vv + uu -> state slot k+1 (off critical path)
                    if s == 0 and (k + 1) % CHUNK == 0 and \
                            (k + 1) // CHUNK >= 3:
                        cp = (k + 1) // CHUNK
                        gp.wait_ge(dma_o[cp % 3], 16 * (cp // 3))
                    nc.gpsimd.tensor_add(
                        slot_ap(k + 1, s, slice(0, 2 * H)),
                        half(vv, s), half(uu, s)).then_inc(sem_g[s])

    return nc, nch


def _get_program(t_steps):
    if t_steps not in _PROGRAM_CACHE:
        _PROGRAM_CACHE[t_steps] = _build_program(t_steps)
    return _PROGRAM_CACHE[t_steps]


def _pack_weights(w_ih0, w_hh0, b_ih0, b_hh0, w_ih1, w_hh1, b_ih1, b_hh1):
    """Column layout (116): z0@0 z1@10 r0@32 r1@42 hn0@64 hn1@74
    xn0@96 xn1@106; rows = [h0(10); h1(10); pad; x@32]."""
    w_rec = np.zeros((SR, 116), np.float32)
    # rows 0:10 contract with h0
    w_rec[0:10, 32:42] = w_hh0[0:10, :].T      # r0
    w_rec[0:10, 0:10] = w_hh0[10:20, :].T      # z0
    w_rec[0:10, 64:74] = w_hh0[20:30, :].T     # hn0
    w_rec[0:10, 42:52] = w_ih1[0:10, :].T      # r1 (layer1 input side)
    w_rec[0:10, 10:20] = w_ih1[10:20, :].T     # z1
    w_rec[0:10, 106:116] = w_ih1[20:30, :].T   # xn1
    # rows 10:20 contract with h1
    w_rec[10:20, 42:52] = w_hh1[0:10, :].T     # r1
    w_rec[10:20, 10:20] = w_hh1[10:20, :].T    # z1
    w_rec[10:20, 74:84] = w_hh1[20:30, :].T    # hn1
    # row XROW contracts with x_t (rank-1 layer0 input side)
    w_rec[XROW, 32:42] = w_ih0[0:10, 0]          # r0
    w_rec[XROW, 0:10] = w_ih0[10:20, 0]          # z0
    w_rec[XROW, 96:106] = w_ih0[20:30, 0]        # xn0

    bvec = np.zeros((116, 1), np.float32)
    bvec[32:42, 0] = b_ih0[0:10] + b_hh0[0:10]     # r0
    bvec[42:52, 0] = b_ih1[0:10] + b_hh1[0:10]     # r1
    bvec[0:10, 0] = b_ih0[10:20] + b_hh0[10:20]    # z0
    bvec[10:20, 0] = b_ih1[10:20] + b_hh1[10:20]   # z1
    bvec[64:74, 0] = b_hh0[20:30]                  # hn0 (inside r*)
    bvec[74:84, 0] = b_hh1[20:30]                  # hn1
    bvec[96:106, 0] = b_ih0[20:30]                 # xn0
    bvec[106:116, 0] = b_ih1[20:30]                # xn1
    return w_rec.astype(F16), bvec


OUTPUT_NAMES = ["h2"]


def _make_in_maps(x, inputs, t_steps):
    nstep, nslot, nch = _dims(t_steps)
    w_rec, bvec = _pack_weights(
        np.asarray(inputs["w_ih0"], np.float32),
        np.asarray(inputs["w_hh0"], np.float32),
        np.asarray(inputs["b_ih0"], np.float32),
        np.asarray(inputs["b_hh0"], np.float32),
        np.asarray(inputs["w_ih1"], np.float32),
        np.asarray(inputs["w_hh1"], np.float32),
        np.asarray(inputs["b_ih1"], np.float32),
        np.asarray(inputs["b_hh1"], np.float32))
    in_maps = []
    for c in range(NCORES):
        xc = x[c * BL:(c + 1) * BL, :, 0]          # [BL, t]
        xt = np.zeros((nch * CHUNK, BL), np.float32)
        xt[:t_steps, :] = xc.T
        in_maps.append({
            "xt": xt.reshape(nch, CHUNK * BL).astype(F16),
            "w_rec": w_rec, "bvec": bvec,
        })
    return in_maps


def _postprocess_core(outs, inputs, t_steps):
    """outs: dict of this core's output tensors -> [BL, t, 1] block."""
    _, _, nch = _dims(t_steps)
    w_lin = np.asarray(inputs["w_lin"], np.float32)
    b_lin = np.asarray(inputs["b_lin"], np.float32)
    h2 = np.asarray(outs["h2"], np.float32)    # [nch, H, CHUNK*BL]
    # slot s holds layer-1 state after macro-step s-1, which processed
    # layer-1 timestep t' = s-2  =>  h2[b, t] = slot t+2
    arr = h2.reshape(nch, H, CHUNK, BL)
    arr = arr.transpose(3, 0, 2, 1).reshape(BL, nch * CHUNK, H)
    h2_bt = arr[:, 2:t_steps + 2, :]           # [BL, t, H]
    out = np.empty((BL, t_steps, 1), np.float32)
    out[:, :, 0] = h2_bt @ w_lin[0, :] + b_lin[0]
    return out


def run(x, w_ih0, w_hh0, b_ih0, b_hh0, w_ih1, w_hh1, b_ih1, b_hh1,
        w_lin, b_lin, t_steps=T, trace=False):
    from concourse.bass_utils import run_bass_kernel_spmd

    nc, nch = _get_program(t_steps)

    x = np.asarray(x, np.float32)
    inputs = dict(w_ih0=w_ih0, w_hh0=w_hh0, b_ih0=b_ih0, b_hh0=b_hh0,
                  w_ih1=w_ih1, w_hh1=w_hh1, b_ih1=b_ih1, b_hh1=b_hh1,
                  w_lin=w_lin, b_lin=b_lin)
    in_maps = _make_in_maps(x, inputs, t_steps)

    res = run_bass_kernel_spmd(nc, in_maps, list(range(NCORES)), trace=trace)

    out = np.empty((B, t_steps, 1), np.float32)
    for c in range(NCORES):
        out[c * BL:(c + 1) * BL] = _postprocess_core(
            res.results[c], inputs, t_steps)
    return out, res


def kernel(x, w_ih0, w_hh0, b_ih0, b_hh0, w_ih1, w_hh1, b_ih1, b_hh1,
           w_lin, b_lin):
    out, _ = run(x, w_ih0, w_hh0, b_ih0, b_hh0, w_ih1, w_hh1, b_ih1, b_hh1,
                 w_lin, b_lin)
    return out


# revision 36
# speedup vs baseline: 1.0451x; 1.0237x over previous
"""Trainium2 Bass kernel for a 2-layer GRU (H=10) + linear head.

Strategy (pure data parallel, 8 cores):
  - Shard batch B=1024 -> 128 per core; replicate the tiny weights.
  - Per core, the T=2048 recurrence runs as one fused chain covering BOTH
    GRU layers (layer 1 lags layer 0 by one step, software-pipelined).
  - fp16 state/weights: PE runs at 1 cycle/row (fp32 is 4).
  - PSUM-accumulated recurrence: h' = vv + uu  (vv = n*(1-z), uu = h*z,
    both on Pool), and W @ h' = W@vv + W@uu, so the step-(k+1)
    preactivations are built by THREE accumulating matmuls into one PSUM
    bank: mm_x (rank-1 x side, start=True), mm_uu, mm_vv.  The h'
    materialization (Pool add into the state chunk tile) is OFF the
    critical path - the next matmul fires as soon as vv lands.
  - TWO staggered streams per core (batch cols 0:64 / 64:128).  Each
    stream has its own PSUM banks + semaphores and runs its latency
    chain independently; engines interleave A/B instructions, and the
    narrower (64-col) instructions shorten the chain itself.
  - hb = hn + b_hn is prestaged on DVE while sigma runs (gated only on
    the matmul), so the post-sigma DVE work is tt = hb*r (all-SBUF)
    then narg = (xn + b_in) + tt.  Steady-state chain ~1.42us/step:
    sigma -> tt,narg (DVE) -> tanh -> vv (Pool) -> mm_vv (PE) ->
    sigma, each hop paying the 100ns semaphore delay.
  - PSUM layout per stream ([116, 64] fp32): z 0:20 | r 32:52 |
    hn 64:84 | xn 96:116 (20-row blocks at 32-row quadrant bases);
    narg in a shared [20, 128] bank.  NOTE: hardware rejects PSUM
    accumulation groups that interleave with other banks' groups and
    engine ops whose two SBUF inputs have different base partitions -
    CoreSim checks neither.
  - h' states accumulate in [33, CHUNK*128] fp16 chunk tiles (rows 0:20
    h0|h1, row 32 = x_t DMA'd in); layer-1 rows DMA out per chunk; the
    final linear head (10 -> 1) runs on host.
"""

import numpy as np
F16 = np.float16

H = 10
B = 1024
T = 2048
NCORES = 8
BL = B // NCORES  # 128 batch rows per core
NS = 3            # staggered streams per core
_W = [BL // NS + (1 if i < BL % NS else 0) for i in range(NS)]
_C0 = [sum(_W[:i]) for i in range(NS)]    # col offset of stream i
CHUNK = 8         # time slots per SBUF state chunk
SR = 33           # state rows: h0(10) | h1(10) | pad | x @ 32
XROW = 32         # x row partition (matmul base partition must be 0/32/64)

_PROGRAM_CACHE = {}


def _dims(t_steps):
    nstep = t_steps + 1          # macro-steps 0..t (layer1 lags by one)
    nslot = nstep + 1            # state slots
    nch = (nslot + CHUNK - 1) // CHUNK
    return nstep, nslot, nch


def _build_program(t_steps):
    from contextlib import ExitStack

    import concourse.bass as bass
    import concourse.mybir as mybir

    fp32 = mybir.dt.float32
    f16 = mybir.dt.float16
    Alu = mybir.AluOpType
    Act = mybir.ActivationFunctionType

    nstep, nslot, nch = _dims(t_steps)

    nc = bass.Bass()

    x_d = nc.declare_dram_parameter("xt", [nch, CHUNK * BL], f16,
                                    isOutput=False)
    wrec_d = nc.declare_dram_parameter("w_rec", [SR, 116], f16,
                                       isOutput=False)
    bvec_d = nc.declare_dram_parameter("bvec", [116, 1], fp32,
                                       isOutput=False)
    h2_d = nc.declare_dram_parameter("h2", [nch, H, CHUNK * BL], f16,
                                     isOutput=True)

    ctx = ExitStack()
    sb = lambda shape, name, dt=fp32: ctx.enter_context(
        nc.sbuf_tensor(name, shape, dt))
    ps = lambda shape, name: ctx.enter_context(
        nc.psum_tensor(name, shape, fp32))
    sem = lambda name: ctx.enter_context(nc.semaphore(name))

    wh_raw = sb([SR, 116], "wh_raw", f16)
    bv_raw = sb([116, 1], "bv_raw")
    w_h = sb([SR, 116], "w_h", f16)   # rows 0:20 = W (h side), row 32 = w_x
    bv = sb([116, 1], "bv")
    b_hn2 = sb([52, 1], "b_hn2")   # b_hh n-gate at rows 32:52 (matches r)
    b_in2 = sb([20, 1], "b_in2")   # b_ih n-gate at rows 0:20 (matches tt)
    # shared [., BL] tiles; stream s uses cols s*HB:(s+1)*HB
    srz = sb([52, BL], "srz", f16)
    hb = sb([52, BL], "hb", f16)   # hn + b_hn staged at rows 32:52
    tt = sb([20, BL], "tt", f16)
    sn = sb([20, BL], "sn", f16)
    omz = sb([20, BL], "omz", f16)
    uu = sb([20, BL], "uu", f16)
    vv = sb([20, BL], "vv", f16)
    state = [sb([SR, CHUNK * BL], f"state{i}", f16) for i in range(3)]
    # P banks per stream (accumulation groups are per-bank); narg shared
    Pb = [[ps([116, _W[s]], f"P{s}{i}") for i in range(2)]
          for s in range(NS)]
    nargb = ps([20, BL], "narg")

    def Pv(s, i, rows=slice(0, 116)):
        return Pb[s][i][rows, :]

    def nargv(s):
        return nargb[:, _C0[s]:_C0[s] + _W[s]]

    sem_d = [sem(f"sem_d{s}") for s in range(NS)]  # DVE: memset + narg
    sem_a = [sem(f"sem_a{s}") for s in range(NS)]  # ACT: 2/step
    sem_p = [sem(f"sem_p{s}") for s in range(NS)]  # PE: P(k) complete
    sem_g = [sem(f"sem_g{s}") for s in range(NS)]  # Pool: uu, vv, h'
    dma_w = sem("dma_w")
    dma_x = [sem(f"dma_x{i}") for i in range(3)]  # x chunks, by c%3
    dma_o = [sem(f"dma_o{i}") for i in range(3)]  # h2 out, by c%3

    def slot_ap(s, strm, rows=slice(0, SR)):
        c0 = (s % CHUNK) * BL + _C0[strm]
        return state[(s // CHUNK) % 3][rows, c0:c0 + _W[strm]]

    def half(t, strm, rows=None):
        sl = slice(_C0[strm], _C0[strm] + _W[strm])
        if rows is None:
            return t[:, sl]
        return t[rows, sl]

    with nc.Block() as block:

        @block.sync
        def _(sp):
            sp.dma_start(wh_raw[:, :], wrec_d[:]).then_inc(dma_w, 16)
            sp.dma_start(bv_raw[:, :], bvec_d[:]).then_inc(dma_w, 16)
            # x for chunks 0..2 straight into state row XROW
            for c in range(min(3, nch)):
                sp.dma_start(state[c][XROW:XROW + 1, :], x_d[c]).then_inc(
                    dma_x[c], 16)
            for c in range(nch):
                # stream out chunk c once its last h' lands (both streams)
                last_k = min(CHUNK * c + CHUNK - 2, nstep - 1)
                for s in range(NS):
                    sp.wait_ge(sem_g[s], 3 * last_k + 3)
                sp.dma_start(h2_d[c], state[c % 3][H:2 * H, :]).then_inc(
                    dma_o[c % 3], 16)
                # refill x row of tile (c+3) once chunk c's matmuls done
                if c + 3 < nch:
                    for s in range(NS):
                        sp.wait_ge(sem_p[s], CHUNK * (c + 1))
                    sp.dma_start(state[c % 3][XROW:XROW + 1, :],
                                 x_d[c + 3]).then_inc(dma_x[c % 3], 16)

        @block.tensor
        def _(pe):
            # iter k builds P_s(k) = w_x@x(k) + W@uu(k-1) + W@vv(k-1)
            for k in range(nstep):
                if k % CHUNK == 0:
                    c = k // CHUNK
                    pe.wait_ge(dma_x[c % 3], 16 * (c // 3 + 1))
                for s in range(NS):
                    pe.wait_ge(sem_d[s], max(1, k))
                    mm = nc.tensor.matmul(
                        Pv(s, k % 2), w_h[XROW:XROW + 1, :],
                        slot_ap(k, s, slice(XROW, XROW + 1)),
                        start=True, stop=(k == 0))
                    if k == 0:
                        mm.then_inc(sem_p[s])
                        continue
                    pe.wait_ge(sem_g[s], 3 * (k - 1) + 1)
                    nc.tensor.matmul(Pv(s, k % 2), w_h[0:2 * H, :],
                                     half(uu, s), start=False, stop=False)
                    pe.wait_ge(sem_g[s], 3 * (k - 1) + 2)
                    nc.tensor.matmul(Pv(s, k % 2), w_h[0:2 * H, :],
                                     half(vv, s), start=False,
                                     stop=True).then_inc(sem_p[s])

        @block.scalar
        def _(act):
            for k in range(nstep):
                for s in range(NS):
                    act.wait_ge(sem_p[s], k + 1)
                    nc.scalar.activation(half(srz, s),
                                         Pv(s, k % 2, slice(0, 52)),
                                         Act.Sigmoid,
                                         bias=bv[0:52, :]).then_inc(sem_a[s])
                for s in range(NS):
                    act.wait_ge(sem_d[s], k + 2)
                    nc.scalar.activation(half(sn, s), nargv(s),
                                         Act.Tanh).then_inc(sem_a[s])

        @block.vector
        def _(dve):
            dve.wait_ge(dma_w, 32)
            nc.vector.tensor_copy(w_h[:, :], wh_raw[:, :])
            nc.vector.tensor_copy(bv[:, :], bv_raw[:, :])
            nc.vector.tensor_copy(b_hn2[32:52, :], bv_raw[64:84, :])
            nc.vector.tensor_copy(b_in2[:, :], bv_raw[96:116, :])
            # h(0) = 0 (slot 0 only; h'(0) full-writes slot 1)
            for s in range(NS):
                nc.vector.memset(
                    state[0][0:2 * H, _C0[s]:_C0[s] + _W[s]],
                    0.0).then_inc(sem_d[s])
            for s in range(NS):
                dve.wait_ge(sem_p[s], 1)
                nc.vector.tensor_scalar_add(
                    half(hb, s, slice(32, 52)),
                    Pv(s, 0, slice(64, 84)), b_hn2[32:52, :])
            for k in range(nstep):
                for s in range(NS):
                    dve.wait_ge(sem_a[s], 2 * k + 1)
                    # t = hb * r  (hb prestaged last iter; both SBUF @32:52)
                    nc.vector.tensor_mul(
                        half(tt, s), half(hb, s, slice(32, 52)),
                        half(srz, s, slice(32, 52)))
                    # n_arg = (xn + b_in) + t
                    nc.vector.scalar_tensor_tensor(
                        nargv(s), Pv(s, k % 2, slice(96, 116)), b_in2[:, :],
                        half(tt, s), op0=Alu.add,
                        op1=Alu.add).then_inc(sem_d[s])
                # prestage hb(k+1) = hn + b_hn while sigma(k+1) runs
                if k + 1 < nstep:
                    for s in range(NS):
                        dve.wait_ge(sem_p[s], k + 2)
                        nc.vector.tensor_scalar_add(
                            half(hb, s, slice(32, 52)),
                            Pv(s, (k + 1) % 2, slice(64, 84)),
                            b_hn2[32:52, :])

        @block.gpsimd
        def _(gp):
            # vv rows 10:20 must read as 0 at k=0 (layer-1 not live yet)
            nc.gpsimd.memset(vv[:, :], 0.0)
            for k in range(nstep):
                for s in range(NS):
                    gp.wait_ge(sem_a[s], 2 * k + 1)
                    # omz = 1 - z ; u = h * z   (z @ rows 0:20)
                    nc.gpsimd.tensor_scalar(half(omz, s),
                                            half(srz, s, slice(0, 20)),
                                            -1.0, 1.0,
                                            op0=Alu.mult, op1=Alu.add)
                    nc.gpsimd.tensor_mul(
                        half(uu, s), slot_ap(k, s, slice(0, 2 * H)),
                        half(srz, s, slice(0, 20))).then_inc(sem_g[s])
                for s in range(NS):
                    gp.wait_ge(sem_a[s], 2 * k + 2)
                    # vv = n * (1 - z)
                    if k == 0:
                        nc.gpsimd.tensor_mul(
                            half(vv, s, slice(0, H)),
                            half(sn, s, slice(0, H)),
                            half(omz, s, slice(0, H))).then_inc(sem_g[s])
                    else:
                        nc.gpsimd.tensor_mul(
                            half(vv, s), half(sn, s),
                            half(omz, s)).then_inc(sem_g[s])
                    # h' = vv + uu -> state slot k+1 (off critical path)
                    if s == 0 and (k + 1) % CHUNK == 0 and \
                            (k + 1) // CHUNK >= 3:
                        cp = (k + 1) // CHUNK
                        gp.wait_ge(dma_o[cp % 3], 16 * (cp // 3))
                    nc.gpsimd.tensor_add(
                        slot_ap(k + 1, s, slice(0, 2 * H)),
                        half(vv, s), half(uu, s)).then_inc(sem_g[s])

    return nc, nch


def _get_program(t_steps):
    if t_steps not in _PROGRAM_CACHE:
        _PROGRAM_CACHE[t_steps] = _build_program(t_steps)
    return _PROGRAM_CACHE[t_steps]


def _pack_weights(w_ih0, w_hh0, b_ih0, b_hh0, w_ih1, w_hh1, b_ih1, b_hh1):
    """Column layout (116): z0@0 z1@10 r0@32 r1@42 hn0@64 hn1@74
    xn0@96 xn1@106; rows = [h0(10); h1(10); pad; x@32]."""
    w_rec = np.zeros((SR, 116), np.float32)
    # rows 0:10 contract with h0
    w_rec[0:10, 32:42] = w_hh0[0:10, :].T      # r0
    w_rec[0:10, 0:10] = w_hh0[10:20, :].T      # z0
    w_rec[0:10, 64:74] = w_hh0[20:30, :].T     # hn0
    w_rec[0:10, 42:52] = w_ih1[0:10, :].T      # r1 (layer1 input side)
    w_rec[0:10, 10:20] = w_ih1[10:20, :].T     # z1
    w_rec[0:10, 106:116] = w_ih1[20:30, :].T   # xn1
    # rows 10:20 contract with h1
    w_rec[10:20, 42:52] = w_hh1[0:10, :].T     # r1
    w_rec[10:20, 10:20] = w_hh1[10:20, :].T    # z1
    w_rec[10:20, 74:84] = w_hh1[20:30, :].T    # hn1
    # row XROW contracts with x_t (rank-1 layer0 input side)
    w_rec[XROW, 32:42] = w_ih0[0:10, 0]          # r0
    w_rec[XROW, 0:10] = w_ih0[10:20, 0]          # z0
    w_rec[XROW, 96:106] = w_ih0[20:30, 0]        # xn0

    bvec = np.zeros((116, 1), np.float32)
    bvec[32:42, 0] = b_ih0[0:10] + b_hh0[0:10]     # r0
    bvec[42:52, 0] = b_ih1[0:10] + b_hh1[0:10]     # r1
    bvec[0:10, 0] = b_ih0[10:20] + b_hh0[10:20]    # z0
    bvec[10:20, 0] = b_ih1[10:20] + b_hh1[10:20]   # z1
    bvec[64:74, 0] = b_hh0[20:30]                  # hn0 (inside r*)
    bvec[74:84, 0] = b_hh1[20:30]                  # hn1
    bvec[96:106, 0] = b_ih0[20:30]                 # xn0
    bvec[106:116, 0] = b_ih1[20:30]                # xn1
    return w_rec.astype(F16), bvec


OUTPUT_NAMES = ["h2"]


def _make_in_maps(x, inputs, t_steps):
    nstep, nslot, nch = _dims(t_steps)
    w_rec, bvec = _pack_weights(
        np.asarray(inputs["w_ih0"], np.float32),
        np.asarray(inputs["w_hh0"], np.float32),
        np.asarray(inputs["b_ih0"], np.float32),
        np.asarray(inputs["b_hh0"], np.float32),
        np.asarray(inputs["w_ih1"], np.float32),
        np.asarray(inputs["w_hh1"], np.float32),
        np.asarray(inputs["b_ih1"], np.float32),
        np.asarray(inputs["b_hh1"], np.float32))
    in_maps = []
    for c in range(NCORES):
        xc = x[c * BL:(c + 1) * BL, :, 0]          # [BL, t]
        xt = np.zeros((nch * CHUNK, BL), np.float32)
        xt[:t_steps, :] = xc.T
        in_maps.append({
            "xt": xt.reshape(nch, CHUNK * BL).astype(F16),
            "w_rec": w_rec, "bvec": bvec,
        })
    return in_maps


def _postprocess_core(outs, inputs, t_steps):
    """outs: dict of this core's output tensors -> [BL, t, 1] block."""
    _, _, nch = _dims(t_steps)
    w_lin = np.asarray(inputs["w_lin"], np.float32)
    b_lin = np.asarray(inputs["b_lin"], np.float32)
    h2 = np.asarray(outs["h2"], np.float32)    # [nch, H, CHUNK*BL]
    # slot s holds layer-1 state after macro-step s-1, which processed
    # layer-1 timestep t' = s-2  =>  h2[b, t] = slot t+2
    arr = h2.reshape(nch, H, CHUNK, BL)
    arr = arr.transpose(3, 0, 2, 1).reshape(BL, nch * CHUNK, H)
    h2_bt = arr[:, 2:t_steps + 2, :]           # [BL, t, H]
    out = np.empty((BL, t_steps, 1), np.float32)
    out[:, :, 0] = h2_bt @ w_lin[0, :] + b_lin[0]
    return out


def run(x, w_ih0, w_hh0, b_ih0, b_hh0, w_ih1, w_hh1, b_ih1, b_hh1,
        w_lin, b_lin, t_steps=T, trace=False):
    from concourse.bass_utils import run_bass_kernel_spmd

    nc, nch = _get_program(t_steps)

    x = np.asarray(x, np.float32)
    inputs = dict(w_ih0=w_ih0, w_hh0=w_hh0, b_ih0=b_ih0, b_hh0=b_hh0,
                  w_ih1=w_ih1, w_hh1=w_hh1, b_ih1=b_ih1, b_hh1=b_hh1,
                  w_lin=w_lin, b_lin=b_lin)
    in_maps = _make_in_maps(x, inputs, t_steps)

    res = run_bass_kernel_spmd(nc, in_maps, list(range(NCORES)), trace=trace)

    out = np.empty((B, t_steps, 1), np.float32)
    for c in range(NCORES):
        out[c * BL:(c + 1) * BL] = _postprocess_core(
            res.results[c], inputs, t_steps)
    return out, res


def kernel(x, w_ih0, w_hh0, b_ih0, b_hh0, w_ih1, w_hh1, b_ih1, b_hh1,
           w_lin, b_lin):
    out, _ = run(x, w_ih0, w_hh0, b_ih0, b_hh0, w_ih1, w_hh1, b_ih1, b_hh1,
                 w_lin, b_lin)
    return out
